# revision 48
# baseline (speedup 1.0000x reference)
"""Trainium2 Bass kernel for the additive-attention layer.

Math (per batch b):
    pre[s, h]   = enc[b] @ W2 + hidden[b] @ W1 + b_attn      (W1=W_attn[:H], W2=W_attn[H:])
    energy      = tanh(pre)
    scores[s]   = energy @ w_v (+ b_v, irrelevant: softmax is shift-invariant)
    attn        = softmax(scores)
    context     = attn @ enc[b]

Distribution: data-parallel over batch, 4 batches per core, no collectives.

Host-side prep (cheap, numpy):
  - enc is quantized to int8 with one global scale g (max/127). int8 halves
    the bytes shipped to the device vs bf16; g is folded into W2 on the
    dequant path so the device never needs a data-dependent immediate.
  - hproj[b] = hidden[b] @ W1 + b_attn computed on host (tiny), so W1 and
    hidden never ship.
  - All device tensors are pre-laid-out so every DMA lands as large
    contiguous per-partition descriptors (8-32 KB), not 1 KB strided runs.

Device dataflow per (batch, s-chunk of 512):
  - etq int8 tile (128p, 16k x 512s) arrives via HWDGE DMA (contiguous
    8KB/partition descriptors).
  - upcast etq -> et bf16 per k-tile, split across ScalarE (Identity
    activation) and VectorE (tensor_copy) - exact, values are ints in
    [-127,127]. (GpSimd copies and SWDGE casting DMAs both run ~35G elem/s
    on the Q7s - measured - so the cast is spread over the fast engines.)
  - TensorE: psum[h128, s512] += w2[d128, h128].T @ et[d128, s512] (16 d-tiles)
    (w2 = dequantized from int8 once at start, g*gw folded in.)
  - ScalarE: energy = tanh(psum + hproj[b]) -> SBUF bf16 (bias = per-partition)
  - TensorE: scores_psum[1, s512] += w_v[h128, 1].T @ energy  (8 h-tiles)
  - ScalarE: p = exp(scores) -> attn row (bf16), accum_out = chunk denominator
  - GpSimd: broadcast p across 128 partitions (staged through an offset-0
    tile; broadcast from an offset slice is ~20x slower)
  - VectorE: fused ctx partials: one stride-0-broadcast tensor_mul
    (128,16,512) * p + one tensor_reduce -> (128,16) chunk column
Per-batch finalize (overlaps the next batch): denominator reduce -> DMA out;
raw bf16 attn rows and unnormalized ctx partial sums DMA out as-is. The
final 1/den (attn) and g/den (ctx) scaling happens on HOST - numerically
identical, and it keeps gpsimd broadcasts and scale ops off the kernel's
critical tail.
"""

import numpy as np
import ml_dtypes
from contextlib import ExitStack

import concourse.bacc as bacc
import concourse.bass as bass
import concourse.tile as tile
import concourse.mybir as mybir
from concourse.bass_utils import run_bass_kernel_spmd

B, S, H = 32, 2048, 1024
D = 2 * H                     # encoder feature dim
NCORES = 8
BPC = B // NCORES             # batches per core
SCH = 512                     # s-chunk (one PSUM bank of fp32)
NCH = S // SCH
NDT = D // 128                # d-tiles (contraction tiles for main matmul)
NHT = H // 128                # h-tiles

BF16 = mybir.dt.bfloat16
F32 = mybir.dt.float32
I8 = mybir.dt.int8

_CACHE = {}


def _build(encp_bufs=3, etp_bufs=3, enp_bufs=12, ppre_bufs=6, psc_bufs=2,
           scr_bufs=1, dve_ct=12):
    nc = bacc.Bacc("TRN2", target_bir_lowering=False, debug=False)

    encq = nc.dram_tensor("encq", (BPC, NCH, 128, NDT * SCH), I8,
                          kind="ExternalInput").ap()
    w2q = nc.dram_tensor("w2q", (128, NDT * H), I8, kind="ExternalInput").ap()
    hproj_in = nc.dram_tensor("hproj", (128, NHT * BPC), F32,
                              kind="ExternalInput").ap()
    wv = nc.dram_tensor("wv", (128, NHT), BF16, kind="ExternalInput").ap()
    # scales, pre-broadcast to 128 partitions on host: [:,0] = g*gw
    gsc = nc.dram_tensor("gsc", (128, 2), F32, kind="ExternalInput").ap()
    # ctx ships UNNORMALIZED (sum of encq*p); attn ships the raw bf16 exp
    # rows; dens ships the per-batch softmax denominators. The final
    # 1/den (and g) scaling happens on host - numerically identical, and it
    # removes a gpsimd broadcast + scale chain from the kernel's tail.
    ctx_out = nc.dram_tensor("ctx", (BPC, D), F32, kind="ExternalOutput").ap()
    attn_out = nc.dram_tensor("attn", (BPC, S), BF16, kind="ExternalOutput").ap()
    dens_out = nc.dram_tensor("dens", (1, BPC), F32, kind="ExternalOutput").ap()

    with tile.TileContext(nc) as tc, ExitStack() as ctx:
        weights = ctx.enter_context(tc.tile_pool(name="weights", bufs=1))
        encp = ctx.enter_context(tc.tile_pool(name="encp", bufs=encp_bufs))
        etp = ctx.enter_context(tc.tile_pool(name="etp", bufs=etp_bufs))
        enp = ctx.enter_context(tc.tile_pool(name="enp", bufs=enp_bufs))
        small = ctx.enter_context(tc.tile_pool(name="small", bufs=1))
        bcp = ctx.enter_context(tc.tile_pool(name="bcp", bufs=2))
        scr = ctx.enter_context(tc.tile_pool(name="scr", bufs=scr_bufs))
        ppre = ctx.enter_context(tc.tile_pool(name="ppre", bufs=ppre_bufs, space="PSUM"))
        psc = ctx.enter_context(tc.tile_pool(name="psc", bufs=psc_bufs, space="PSUM"))

        # --- resident weights ---
        hproj = small.tile([128, NHT, BPC], F32)
        nc.sync.dma_start(out=hproj, in_=hproj_in.rearrange("p (j b) -> p j b", b=BPC))
        wv_sb = small.tile([128, NHT], BF16)
        nc.sync.dma_start(out=wv_sb, in_=wv)
        gsc_sb = small.tile([128, 2], F32)
        nc.sync.dma_start(out=gsc_sb, in_=gsc)
        # dummy ScalarE op: pulls the ~2.7us ACT table load off the
        # W2-dequant critical path (it overlaps the weight DMAs instead)
        warm0 = small.tile([1, 1], F32)
        nc.vector.memset(warm0, 0.0)
        warm1 = small.tile([1, 1], F32)
        nc.scalar.activation(out=warm1, in_=warm0,
                             func=mybir.ActivationFunctionType.Identity)

        # Prefetch the first two enc chunks BEFORE the W2 slice DMAs: all
        # these share the Sync HWDGE FIFO, and the first matmul needs et
        # (0,0) as much as it needs W2 slice 0 - don't queue 2MB of W2
        # ahead of it.
        prefetched = {}
        for pb, pc in ((0, 0), (0, 1)):
            t = encp.tile([128, NDT * SCH], I8, name=f"etqp{pb}{pc}",
                          tag="etq_flat")
            nc.sync.dma_start(out=t, in_=encq[pb, pc])
            prefetched[(pb, pc)] = t

        # W2: int8 DMA, dequant per j-slice (ACT upcast + DVE scale in place)
        # so the first matmul group only waits on slice j=0, not all of W2.
        # Host layout is (j, k, h') contiguous: [p, j*NDT*128 + k*128 + h'].
        ggw = gsc_sb[:, 0:1]
        w2q_sb = small.tile([128, NDT * H], I8, name="w2q_sb", tag="w2q_sb")
        w2_sb_flat = weights.tile([128, NDT * H], BF16)
        w2q_j = w2q_sb.rearrange("p (j x) -> p j x", j=NHT)
        w2_j = w2_sb_flat.rearrange("p (j x) -> p j x", j=NHT)
        JSL = NDT * 128
        for j in range(NHT):
            nc.scalar.dma_start(
                out=w2q_j[:, j, :], in_=w2q[:, j * JSL:(j + 1) * JSL]
            )
            nc.scalar.activation(
                out=w2_j[:, j, :],
                in_=w2q_j[:, j, :],
                func=mybir.ActivationFunctionType.Identity,
            )
            nc.vector.tensor_scalar_mul(w2_j[:, j, :], w2_j[:, j, :], ggw)
        # matmul view: lhsT for (j, k) = w2_sb[:, j, k, :]
        w2_sb = w2_sb_flat.rearrange("p (j k h) -> p j k h", j=NHT, k=NDT)

        # --- persistent accumulators ---
        # (engines can't address partition offsets 1..3, so per-batch rows
        # live as separate partition-0 tiles)
        attn_rows = [
            small.tile([1, S], BF16, name=f"attnrow{b}", tag=f"attnrow{b}")
            for b in range(BPC)
        ]
        denp = small.tile([1, BPC * NCH], F32)
        # context partials: column layout (b, k, c)
        ctxp = small.tile([128, BPC * NDT * NCH], F32)

        # --- main loop ---
        for b in range(BPC):
            for c in range(NCH):
                etq_flat = prefetched.pop((b, c), None)
                if etq_flat is None:
                    etq_flat = encp.tile([128, NDT * SCH], I8, name="etq_flat",
                                         tag="etq_flat")
                    nc.sync.dma_start(out=etq_flat, in_=encq[b, c])
                etq = etq_flat.rearrange("p (k s) -> p k s", s=SCH)
                et_flat = etp.tile([128, NDT * SCH], BF16)
                et = et_flat.rearrange("p (k s) -> p k s", s=SCH)
                # upcast int8 -> bf16 split across DVE and ACT. Chunk (0,0)
                # is special: ScalarE is busy with W2 slice upcasts and the
                # DVE FIFO holds the W2 scale-muls, so its first k-tiles go
                # to the otherwise-idle GpSimd (slow per-tile but off the
                # critical path) and the rest to DVE.
                if (b, c) == (0, 0):
                    for k in range(NDT):
                        if k < 4:
                            nc.gpsimd.tensor_copy(et[:, k, :], etq[:, k, :])
                        else:
                            nc.vector.tensor_copy(et[:, k, :], etq[:, k, :])
                else:
                    for k in range(NDT):
                        if k < dve_ct:
                            nc.vector.tensor_copy(et[:, k, :], etq[:, k, :])
                        else:
                            nc.scalar.activation(
                                out=et[:, k, :],
                                in_=etq[:, k, :],
                                func=mybir.ActivationFunctionType.Identity,
                            )

                energies = []
                for j in range(NHT):
                    pp = ppre.tile([128, SCH], F32)
                    for k in range(NDT):
                        nc.tensor.matmul(
                            pp,
                            w2_sb[:, j, k, :],
                            et[:, k, :],
                            start=(k == 0),
                            stop=(k == NDT - 1),
                        )
                    en = enp.tile([128, SCH], BF16)
                    nc.scalar.activation(
                        out=en,
                        in_=pp,
                        func=mybir.ActivationFunctionType.Tanh,
                        bias=hproj[:, j, b:b + 1],
                        scale=1.0,
                    )
                    energies.append(en)

                ps = psc.tile([1, SCH], F32)
                for j in range(NHT):
                    nc.tensor.matmul(
                        ps,
                        wv_sb[:, j:j + 1],
                        energies[j],
                        start=(j == 0),
                        stop=(j == NHT - 1),
                    )

                prow = attn_rows[b][0:1, c * SCH:(c + 1) * SCH]
                dcol = b * NCH + c
                nc.scalar.activation(
                    out=prow,
                    in_=ps,
                    func=mybir.ActivationFunctionType.Exp,
                    accum_out=denp[0:1, dcol:dcol + 1],
                )

                # broadcast p across partitions. NOTE: partition_broadcast
                # from an offset slice is ~20x slower on the Q7s than from
                # an offset-0 tile, so stage through a dedicated tile.
                prow_bf = bcp.tile([1, SCH], BF16)
                nc.vector.tensor_copy(prow_bf, prow)
                pbc = bcp.tile([128, SCH], BF16)
                nc.gpsimd.partition_broadcast(pbc, prow_bf)

                # fused ctx partials: one broadcast-mul (stride-0 AP over k)
                # + one reduce (128,16,512)->(128,16) instead of 16 op pairs
                ctxp3 = ctxp.rearrange("p (x c) -> p x c", c=NCH)
                prod3 = scr.tile([128, NDT, SCH], BF16)
                if (b, c) != (BPC - 1, NCH - 1):
                    pbc_b = bass.AP(
                        pbc.tensor, pbc.offset,
                        [list(pbc.ap[0]), [0, NDT], [1, SCH]],
                    )
                    nc.vector.tensor_mul(prod3, et, pbc_b)
                    nc.vector.tensor_reduce(
                        ctxp3[:, b * NDT:(b + 1) * NDT, c:c + 1],
                        prod3,
                        axis=mybir.AxisListType.X,
                        op=mybir.AluOpType.add,
                    )
                else:
                    # LAST chunk sits on the kernel tail: split the work by
                    # k-halves across DVE and the now-idle ScalarE so the two
                    # run concurrently after the final exp.
                    HK = NDT // 2
                    pbc_h = bass.AP(
                        pbc.tensor, pbc.offset,
                        [list(pbc.ap[0]), [0, HK], [1, SCH]],
                    )
                    nc.vector.tensor_mul(prod3[:, 0:HK, :], et[:, 0:HK, :], pbc_h)
                    nc.vector.tensor_mul(prod3[:, HK:NDT, :], et[:, HK:NDT, :], pbc_h)
                    scrap = scr.tile([128, SCH], BF16, name="scrap", tag="scrap")
                    for k in range(HK):
                        nc.scalar.activation(
                            out=scrap,
                            in_=prod3[:, k, :],
                            func=mybir.ActivationFunctionType.Identity,
                            accum_out=ctxp3[:, b * NDT + k, c:c + 1],
                        )
                    nc.vector.tensor_reduce(
                        ctxp3[:, b * NDT + HK:(b + 1) * NDT, c:c + 1],
                        prod3[:, HK:NDT, :],
                        axis=mybir.AxisListType.X,
                        op=mybir.AluOpType.add,
                    )

            # --- per-batch finalize (overlaps the next batch's chunks) ---
            dent = small.tile([1, 1], F32, name=f"dent{b}", tag="dent")
            nc.vector.tensor_reduce(
                dent,
                denp[0:1, b * NCH:(b + 1) * NCH],
                axis=mybir.AxisListType.X,
                op=mybir.AluOpType.add,
            )
            nc.sync.dma_start(out=dens_out[0:1, b:b + 1], in_=dent)
            nc.scalar.dma_start(out=attn_out[b:b + 1, :], in_=attn_rows[b])

            # reduce context chunk partials for this batch: (128, k, c) -> (128, k)
            ctxr = bcp.tile([128, NDT], F32, name=f"ctxr{b}", tag="ctxr")
            nc.vector.tensor_reduce(
                ctxr,
                ctxp.rearrange("p (x c) -> p x c", c=NCH)[:, b * NDT:(b + 1) * NDT, :],
                axis=mybir.AxisListType.X,
                op=mybir.AluOpType.add,
            )
            nc.sync.dma_start(
                out=ctx_out[b].rearrange("(k p) -> p k", p=128),
                in_=ctxr,
            )

    nc.compile()
    return nc


def _get_nc():
    if "nc" not in _CACHE:
        _CACHE["nc"] = _build()
    return _CACHE["nc"]


def _prep_inputs(hidden, encoder_outputs, W_attn, b_attn, w_v, b_v):
    bf16 = ml_dtypes.bfloat16
    W1 = W_attn[:H]
    W2 = W_attn[H:]

    # global int8 scales
    g = max(float(np.abs(encoder_outputs).max()), 1e-30) / 127.0
    gw = max(float(np.abs(W2).max()), 1e-30) / 127.0
    encq = np.clip(np.rint(encoder_outputs * (1.0 / g)), -127, 127).astype(np.int8)
    w2q8 = np.clip(np.rint(W2 * (1.0 / gw)), -127, 127).astype(np.int8)

    # device layouts
    # w2q: [p, j*NDT*128 + k*128 + h'] = w2q8[k*128 + p, j*128 + h']
    w2q_dev = np.ascontiguousarray(
        w2q8.reshape(NDT, 128, NHT, 128)
        .transpose(1, 2, 0, 3)
        .reshape(128, NDT * H)
    )
    # wv: [p, j] = w_v[j*128 + p]
    wv_dev = np.ascontiguousarray(w_v.reshape(NHT, 128).T).astype(bf16)
    gsc = np.ascontiguousarray(
        np.broadcast_to(np.array([[g * gw, g]], dtype=np.float32), (128, 2))
    )

    hp_all = (hidden @ W1 + b_attn).astype(np.float32)  # (B, H)

    in_maps = []
    for core in range(NCORES):
        sl = slice(core * BPC, (core + 1) * BPC)
        # encq core layout: (BPC, NCH, 128p, NDT*SCH), [b,c,p,k*SCH+s]
        ec = encq[sl].reshape(BPC, NCH, SCH, NDT, 128).transpose(0, 1, 4, 3, 2)
        ec = np.ascontiguousarray(ec.reshape(BPC, NCH, 128, NDT * SCH))
        # hproj: [p, j*BPC + b] = hp[b, j*128 + p]
        hp = hp_all[sl].T.reshape(NHT, 128, BPC).transpose(1, 0, 2)
        hp = np.ascontiguousarray(hp.reshape(128, NHT * BPC))
        in_maps.append(
            {
                "encq": ec,
                "w2q": w2q_dev,
                "hproj": hp,
                "wv": wv_dev,
                "gsc": gsc,
            }
        )
    return in_maps, g


def kernel(hidden, encoder_outputs, W_attn, b_attn, w_v, b_v, _trace=False):
    nc = _get_nc()
    in_maps, g = _prep_inputs(hidden, encoder_outputs, W_attn, b_attn, w_v, b_v)
    res = run_bass_kernel_spmd(
        nc, in_maps, core_ids=list(range(NCORES)), trace=_trace
    )
    # host-side softmax normalization (numerically identical to doing it
    # on device; the denominators ship back alongside the raw outputs)
    ctxs, attns = [], []
    for r in res.results:
        den = r["dens"][0].astype(np.float64)  # (BPC,)
        ctxs.append(r["ctx"] * (g / den)[:, None])
        attns.append(r["attn"].astype(np.float32) / den[:, None])
    context = np.concatenate(ctxs, axis=0).astype(np.float32)
    attn = np.concatenate(attns, axis=0).astype(np.float32)
    if _trace:
        _CACHE["last_results"] = res
    return context, attn


# revision 49
# speedup vs baseline: 1.0148x; 1.0148x over previous
"""Trainium2 Bass kernel for the additive-attention layer.

Math (per batch b):
    pre[s, h]   = enc[b] @ W2 + hidden[b] @ W1 + b_attn      (W1=W_attn[:H], W2=W_attn[H:])
    energy      = tanh(pre)
    scores[s]   = energy @ w_v (+ b_v, irrelevant: softmax is shift-invariant)
    attn        = softmax(scores)
    context     = attn @ enc[b]

Distribution: data-parallel over batch, 4 batches per core, no collectives.

Host-side prep (cheap, numpy):
  - enc is quantized to int8 with one global scale g (max/127). int8 halves
    the bytes shipped to the device vs bf16; g is folded into W2 on the
    dequant path so the device never needs a data-dependent immediate.
  - hproj[b] = hidden[b] @ W1 + b_attn computed on host (tiny), so W1 and
    hidden never ship.
  - All device tensors are pre-laid-out so every DMA lands as large
    contiguous per-partition descriptors (8-32 KB), not 1 KB strided runs.

Device dataflow per (batch, s-chunk of 512):
  - etq int8 tile (128p, 16k x 512s) arrives via HWDGE DMA (contiguous
    8KB/partition descriptors).
  - upcast etq -> et bf16 per k-tile, split across ScalarE (Identity
    activation) and VectorE (tensor_copy) - exact, values are ints in
    [-127,127]. (GpSimd copies and SWDGE casting DMAs both run ~35G elem/s
    on the Q7s - measured - so the cast is spread over the fast engines.)
  - TensorE: psum[h128, s512] += w2[d128, h128].T @ et[d128, s512] (16 d-tiles)
    (w2 = dequantized from int8 once at start, g*gw folded in.)
  - ScalarE: energy = tanh(psum + hproj[b]) -> SBUF bf16 (bias = per-partition)
  - TensorE: scores_psum[1, s512] += w_v[h128, 1].T @ energy  (8 h-tiles)
  - ScalarE: p = exp(scores) -> attn row (bf16), accum_out = chunk denominator
  - GpSimd: broadcast p across 128 partitions (staged through an offset-0
    tile; broadcast from an offset slice is ~20x slower)
  - VectorE: fused ctx partials: one stride-0-broadcast tensor_mul
    (128,16,512) * p + one tensor_reduce -> (128,16) chunk column
Per-batch finalize (overlaps the next batch): denominator reduce -> DMA out;
raw bf16 attn rows and unnormalized ctx partial sums DMA out as-is. The
final 1/den (attn) and g/den (ctx) scaling happens on HOST - numerically
identical, and it keeps gpsimd broadcasts and scale ops off the kernel's
critical tail.
"""

import numpy as np
import ml_dtypes
from contextlib import ExitStack

import concourse.bacc as bacc
import concourse.bass as bass
import concourse.tile as tile
import concourse.mybir as mybir
from concourse.bass_utils import run_bass_kernel_spmd

B, S, H = 32, 2048, 1024
D = 2 * H                     # encoder feature dim
NCORES = 8
BPC = B // NCORES             # batches per core
SCH = 512                     # s-chunk (one PSUM bank of fp32)
NCH = S // SCH
NDT = D // 128                # d-tiles (contraction tiles for main matmul)
NHT = H // 128                # h-tiles

BF16 = mybir.dt.bfloat16
F32 = mybir.dt.float32
I8 = mybir.dt.int8

_CACHE = {}


def _build(encp_bufs=3, etp_bufs=3, enp_bufs=12, ppre_bufs=6, psc_bufs=2,
           scr_bufs=1, dve_ct=12):
    nc = bacc.Bacc("TRN2", target_bir_lowering=False, debug=False)

    encq = nc.dram_tensor("encq", (BPC, NCH, 128, NDT * SCH), I8,
                          kind="ExternalInput").ap()
    w2q = nc.dram_tensor("w2q", (128, NDT * H), I8, kind="ExternalInput").ap()
    hproj_in = nc.dram_tensor("hproj", (128, NHT * BPC), F32,
                              kind="ExternalInput").ap()
    wv = nc.dram_tensor("wv", (128, NHT), BF16, kind="ExternalInput").ap()
    # scales, pre-broadcast to 128 partitions on host: [:,0] = g*gw
    gsc = nc.dram_tensor("gsc", (128, 2), F32, kind="ExternalInput").ap()
    # ctx ships UNNORMALIZED (sum of encq*p); attn ships the raw bf16 exp
    # rows; dens ships the per-batch softmax denominators. The final
    # 1/den (and g) scaling happens on host - numerically identical, and it
    # removes a gpsimd broadcast + scale chain from the kernel's tail.
    ctx_out = nc.dram_tensor("ctx", (BPC, D), F32, kind="ExternalOutput").ap()
    attn_out = nc.dram_tensor("attn", (BPC, S), BF16, kind="ExternalOutput").ap()
    dens_out = nc.dram_tensor("dens", (1, BPC), F32, kind="ExternalOutput").ap()

    with tile.TileContext(nc) as tc, ExitStack() as ctx:
        weights = ctx.enter_context(tc.tile_pool(name="weights", bufs=1))
        encp = ctx.enter_context(tc.tile_pool(name="encp", bufs=encp_bufs))
        etp = ctx.enter_context(tc.tile_pool(name="etp", bufs=etp_bufs))
        enp = ctx.enter_context(tc.tile_pool(name="enp", bufs=enp_bufs))
        small = ctx.enter_context(tc.tile_pool(name="small", bufs=1))
        bcp = ctx.enter_context(tc.tile_pool(name="bcp", bufs=2))
        scr = ctx.enter_context(tc.tile_pool(name="scr", bufs=scr_bufs))
        ppre = ctx.enter_context(tc.tile_pool(name="ppre", bufs=ppre_bufs, space="PSUM"))
        psc = ctx.enter_context(tc.tile_pool(name="psc", bufs=psc_bufs, space="PSUM"))

        # --- resident weights ---
        hproj = small.tile([128, NHT, BPC], F32)
        nc.sync.dma_start(out=hproj, in_=hproj_in.rearrange("p (j b) -> p j b", b=BPC))
        wv_sb = small.tile([128, NHT], BF16)
        nc.sync.dma_start(out=wv_sb, in_=wv)
        gsc_sb = small.tile([128, 2], F32)
        nc.sync.dma_start(out=gsc_sb, in_=gsc)
        # dummy ScalarE op: pulls the ~2.7us ACT table load off the
        # W2-dequant critical path (it overlaps the weight DMAs instead)
        warm0 = small.tile([1, 1], F32)
        nc.vector.memset(warm0, 0.0)
        warm1 = small.tile([1, 1], F32)
        nc.scalar.activation(out=warm1, in_=warm0,
                             func=mybir.ActivationFunctionType.Identity)

        # Prefetch the first two enc chunks BEFORE the W2 slice DMAs: all
        # these share the Sync HWDGE FIFO, and the first matmul needs et
        # (0,0) as much as it needs W2 slice 0 - don't queue 2MB of W2
        # ahead of it.
        prefetched = {}
        for pb, pc in ((0, 0), (0, 1)):
            t = encp.tile([128, NDT * SCH], I8, name=f"etqp{pb}{pc}",
                          tag="etq_flat")
            nc.sync.dma_start(out=t, in_=encq[pb, pc])
            prefetched[(pb, pc)] = t

        # W2: int8 DMA, dequant per j-slice (ACT upcast + DVE scale in place)
        # so the first matmul group only waits on slice j=0, not all of W2.
        # Host layout is (j, k, h') contiguous: [p, j*NDT*128 + k*128 + h'].
        ggw = gsc_sb[:, 0:1]
        w2q_sb = small.tile([128, NDT * H], I8, name="w2q_sb", tag="w2q_sb")
        w2_sb_flat = weights.tile([128, NDT * H], BF16)
        w2q_j = w2q_sb.rearrange("p (j x) -> p j x", j=NHT)
        w2_j = w2_sb_flat.rearrange("p (j x) -> p j x", j=NHT)
        JSL = NDT * 128
        for j in range(NHT):
            nc.scalar.dma_start(
                out=w2q_j[:, j, :], in_=w2q[:, j * JSL:(j + 1) * JSL]
            )
            nc.scalar.activation(
                out=w2_j[:, j, :],
                in_=w2q_j[:, j, :],
                func=mybir.ActivationFunctionType.Identity,
            )
            nc.vector.tensor_scalar_mul(w2_j[:, j, :], w2_j[:, j, :], ggw)
        # matmul view: lhsT for (j, k) = w2_sb[:, j, k, :]
        w2_sb = w2_sb_flat.rearrange("p (j k h) -> p j k h", j=NHT, k=NDT)

        # --- persistent accumulators ---
        # (engines can't address partition offsets 1..3, so per-batch rows
        # live as separate partition-0 tiles)
        attn_rows = [
            small.tile([1, S], BF16, name=f"attnrow{b}", tag=f"attnrow{b}")
            for b in range(BPC)
        ]
        denp = small.tile([1, BPC * NCH], F32)
        # context partials: column layout (b, k, c)
        ctxp = small.tile([128, BPC * NDT * NCH], F32)

        # --- main loop ---
        for b in range(BPC):
            for c in range(NCH):
                etq_flat = prefetched.pop((b, c), None)
                if etq_flat is None:
                    etq_flat = encp.tile([128, NDT * SCH], I8, name="etq_flat",
                                         tag="etq_flat")
                    nc.sync.dma_start(out=etq_flat, in_=encq[b, c])
                etq = etq_flat.rearrange("p (k s) -> p k s", s=SCH)
                et_flat = etp.tile([128, NDT * SCH], BF16)
                et = et_flat.rearrange("p (k s) -> p k s", s=SCH)
                # upcast int8 -> bf16 split across DVE and ACT. Chunk (0,0)
                # goes all-DVE: ScalarE is busy with W2 slice upcasts then.
                ct = NDT if (b, c) == (0, 0) else dve_ct
                for k in range(NDT):
                    if k < ct:
                        nc.vector.tensor_copy(et[:, k, :], etq[:, k, :])
                    else:
                        nc.scalar.activation(
                            out=et[:, k, :],
                            in_=etq[:, k, :],
                            func=mybir.ActivationFunctionType.Identity,
                        )

                energies = []
                for j in range(NHT):
                    pp = ppre.tile([128, SCH], F32)
                    for k in range(NDT):
                        nc.tensor.matmul(
                            pp,
                            w2_sb[:, j, k, :],
                            et[:, k, :],
                            start=(k == 0),
                            stop=(k == NDT - 1),
                        )
                    en = enp.tile([128, SCH], BF16)
                    nc.scalar.activation(
                        out=en,
                        in_=pp,
                        func=mybir.ActivationFunctionType.Tanh,
                        bias=hproj[:, j, b:b + 1],
                        scale=1.0,
                    )
                    energies.append(en)

                ps = psc.tile([1, SCH], F32)
                for j in range(NHT):
                    nc.tensor.matmul(
                        ps,
                        wv_sb[:, j:j + 1],
                        energies[j],
                        start=(j == 0),
                        stop=(j == NHT - 1),
                    )

                prow = attn_rows[b][0:1, c * SCH:(c + 1) * SCH]
                dcol = b * NCH + c
                nc.scalar.activation(
                    out=prow,
                    in_=ps,
                    func=mybir.ActivationFunctionType.Exp,
                    accum_out=denp[0:1, dcol:dcol + 1],
                )

                # broadcast p across partitions. NOTE: partition_broadcast
                # from an offset slice is ~20x slower on the Q7s than from
                # an offset-0 tile, so stage through a dedicated tile.
                prow_bf = bcp.tile([1, SCH], BF16)
                nc.vector.tensor_copy(prow_bf, prow)
                pbc = bcp.tile([128, SCH], BF16)
                nc.gpsimd.partition_broadcast(pbc, prow_bf)

                # fused ctx partials: one broadcast-mul (stride-0 AP over k)
                # + one reduce (128,16,512)->(128,16) instead of 16 op pairs
                ctxp3 = ctxp.rearrange("p (x c) -> p x c", c=NCH)
                prod3 = scr.tile([128, NDT, SCH], BF16)
                if (b, c) != (BPC - 1, NCH - 1):
                    pbc_b = bass.AP(
                        pbc.tensor, pbc.offset,
                        [list(pbc.ap[0]), [0, NDT], [1, SCH]],
                    )
                    nc.vector.tensor_mul(prod3, et, pbc_b)
                    nc.vector.tensor_reduce(
                        ctxp3[:, b * NDT:(b + 1) * NDT, c:c + 1],
                        prod3,
                        axis=mybir.AxisListType.X,
                        op=mybir.AluOpType.add,
                    )
                else:
                    # LAST chunk sits on the kernel tail: split the work by
                    # k-halves across DVE and the now-idle ScalarE so the two
                    # run concurrently after the final exp.
                    HK = NDT // 2
                    pbc_h = bass.AP(
                        pbc.tensor, pbc.offset,
                        [list(pbc.ap[0]), [0, HK], [1, SCH]],
                    )
                    nc.vector.tensor_mul(prod3[:, 0:HK, :], et[:, 0:HK, :], pbc_h)
                    nc.vector.tensor_mul(prod3[:, HK:NDT, :], et[:, HK:NDT, :], pbc_h)
                    scrap = scr.tile([128, SCH], BF16, name="scrap", tag="scrap")
                    for k in range(HK):
                        nc.scalar.activation(
                            out=scrap,
                            in_=prod3[:, k, :],
                            func=mybir.ActivationFunctionType.Identity,
                            accum_out=ctxp3[:, b * NDT + k, c:c + 1],
                        )
                    nc.vector.tensor_reduce(
                        ctxp3[:, b * NDT + HK:(b + 1) * NDT, c:c + 1],
                        prod3[:, HK:NDT, :],
                        axis=mybir.AxisListType.X,
                        op=mybir.AluOpType.add,
                    )

            # --- per-batch finalize (overlaps the next batch's chunks) ---
            dent = small.tile([1, 1], F32, name=f"dent{b}", tag="dent")
            nc.vector.tensor_reduce(
                dent,
                denp[0:1, b * NCH:(b + 1) * NCH],
                axis=mybir.AxisListType.X,
                op=mybir.AluOpType.add,
            )
            nc.sync.dma_start(out=dens_out[0:1, b:b + 1], in_=dent)
            nc.scalar.dma_start(out=attn_out[b:b + 1, :], in_=attn_rows[b])

            # reduce context chunk partials for this batch: (128, k, c) -> (128, k)
            ctxr = bcp.tile([128, NDT], F32, name=f"ctxr{b}", tag="ctxr")
            nc.vector.tensor_reduce(
                ctxr,
                ctxp.rearrange("p (x c) -> p x c", c=NCH)[:, b * NDT:(b + 1) * NDT, :],
                axis=mybir.AxisListType.X,
                op=mybir.AluOpType.add,
            )
            nc.sync.dma_start(
                out=ctx_out[b].rearrange("(k p) -> p k", p=128),
                in_=ctxr,
            )

    nc.compile()
    return nc


def _get_nc():
    if "nc" not in _CACHE:
        _CACHE["nc"] = _build()
    return _CACHE["nc"]


def _prep_inputs(hidden, encoder_outputs, W_attn, b_attn, w_v, b_v):
    bf16 = ml_dtypes.bfloat16
    W1 = W_attn[:H]
    W2 = W_attn[H:]

    # global int8 scales
    g = max(float(np.abs(encoder_outputs).max()), 1e-30) / 127.0
    gw = max(float(np.abs(W2).max()), 1e-30) / 127.0
    encq = np.clip(np.rint(encoder_outputs * (1.0 / g)), -127, 127).astype(np.int8)
    w2q8 = np.clip(np.rint(W2 * (1.0 / gw)), -127, 127).astype(np.int8)

    # device layouts
    # w2q: [p, j*NDT*128 + k*128 + h'] = w2q8[k*128 + p, j*128 + h']
    w2q_dev = np.ascontiguousarray(
        w2q8.reshape(NDT, 128, NHT, 128)
        .transpose(1, 2, 0, 3)
        .reshape(128, NDT * H)
    )
    # wv: [p, j] = w_v[j*128 + p]
    wv_dev = np.ascontiguousarray(w_v.reshape(NHT, 128).T).astype(bf16)
    gsc = np.ascontiguousarray(
        np.broadcast_to(np.array([[g * gw, g]], dtype=np.float32), (128, 2))
    )

    hp_all = (hidden @ W1 + b_attn).astype(np.float32)  # (B, H)

    in_maps = []
    for core in range(NCORES):
        sl = slice(core * BPC, (core + 1) * BPC)
        # encq core layout: (BPC, NCH, 128p, NDT*SCH), [b,c,p,k*SCH+s]
        ec = encq[sl].reshape(BPC, NCH, SCH, NDT, 128).transpose(0, 1, 4, 3, 2)
        ec = np.ascontiguousarray(ec.reshape(BPC, NCH, 128, NDT * SCH))
        # hproj: [p, j*BPC + b] = hp[b, j*128 + p]
        hp = hp_all[sl].T.reshape(NHT, 128, BPC).transpose(1, 0, 2)
        hp = np.ascontiguousarray(hp.reshape(128, NHT * BPC))
        in_maps.append(
            {
                "encq": ec,
                "w2q": w2q_dev,
                "hproj": hp,
                "wv": wv_dev,
                "gsc": gsc,
            }
        )
    return in_maps, g


def kernel(hidden, encoder_outputs, W_attn, b_attn, w_v, b_v, _trace=False):
    nc = _get_nc()
    in_maps, g = _prep_inputs(hidden, encoder_outputs, W_attn, b_attn, w_v, b_v)
    res = run_bass_kernel_spmd(
        nc, in_maps, core_ids=list(range(NCORES)), trace=_trace
    )
    # host-side softmax normalization (numerically identical to doing it
    # on device; the denominators ship back alongside the raw outputs)
    ctxs, attns = [], []
    for r in res.results:
        den = r["dens"][0].astype(np.float64)  # (BPC,)
        ctxs.append(r["ctx"] * (g / den)[:, None])
        attns.append(r["attn"].astype(np.float32) / den[:, None])
    context = np.concatenate(ctxs, axis=0).astype(np.float32)
    attn = np.concatenate(attns, axis=0).astype(np.float32)
    if _trace:
        _CACHE["last_results"] = res
    return context, attn
